# revision 3
# baseline (speedup 1.0000x reference)
"""Trainium2 Bass kernel for nn_BinaryDense: out = x @ (sum_k sign(b_k)*a_k) + bias.

Shapes (hardcoded): x [4096,4096] f32, b [4,4096,4096] f32, a [4,4096] f32,
bias [4096] f32 -> out [4096,4096] f32.

Strategy: tensor-parallel over the output (units) dim across 8 NeuronCores.
Core c owns O-columns [c*512, (c+1)*512).

Per core:
  1. Build w[:, oc] = sum_k copysign(a[k,oc], b[k,:,oc]) on-chip:
     b arrives bf16 in [I, O_c, K] layout (k innermost), copysign is two
     bitwise DVE ops ((b & 0x8000) | a), the k-sum is a dense innermost
     tensor_reduce.
  2. One bf16 matmul x @ w with fp32 PSUM accumulation:
     lhsT = x^T tiles (host-pretransposed bf16), rhs = w tiles, K=4096.
  3. Add bias on psum eviction, store fp32 [4096, 512].

Host side only reshapes/casts/shards (no math): x^T bf16, b -> [I,O,K] bf16,
a/bias broadcast rows.
"""

import sys

if "/opt/trn_rl_repo" not in sys.path:
    sys.path.insert(0, "/opt/trn_rl_repo")

import numpy as np
import ml_dtypes

BF16 = ml_dtypes.bfloat16

B = 4096   # batch rows of x
I = 4096   # input dim (contraction)
O = 4096   # output dim (sharded)
K = 4      # binary bases
NCORES = 8
OC = O // NCORES   # 512 output cols per core
P = 128

KT = I // P        # 32 k-tiles (contraction)
MT = B // P        # 32 m-tiles (output rows)
M_BLOCK = 4        # m-tiles per psum block


def _build_program():
    import concourse.bass as bass
    import concourse.mybir as mybir
    from concourse import bacc
    from concourse.tile import TileContext

    nc = bacc.Bacc(None, target_bir_lowering=False)

    b_re = nc.declare_dram_parameter("b_re", [I, OC * K], mybir.dt.bfloat16, isOutput=False)
    a_b = nc.declare_dram_parameter("a_b", [P, OC * K], mybir.dt.bfloat16, isOutput=False)
    xT = nc.declare_dram_parameter("xT", [I, B], mybir.dt.bfloat16, isOutput=False)
    bias_b = nc.declare_dram_parameter("bias_b", [P, OC], mybir.dt.float32, isOutput=False)
    out = nc.declare_dram_parameter("out", [B, OC], mybir.dt.float32, isOutput=True)

    with TileContext(nc) as tc:
        with (
            tc.tile_pool(name="const", bufs=1) as const,
            tc.tile_pool(name="bpool", bufs=3) as bpool,
            tc.tile_pool(name="cpool", bufs=3) as cpool,
            tc.tile_pool(name="wpool", bufs=KT) as wpool,
            tc.tile_pool(name="xpool", bufs=6) as xpool,
            tc.tile_pool(name="opool", bufs=3) as opool,
            tc.tile_pool(name="psum", bufs=2, space="PSUM") as psum_pool,
        ):
            a_tile = const.tile([P, OC * K], mybir.dt.bfloat16)
            nc.sync.dma_start(out=a_tile[:], in_=a_b[:, :])
            bias_tile = const.tile([P, OC], mybir.dt.float32)
            nc.sync.dma_start(out=bias_tile[:], in_=bias_b[:, :])
            mask_tile = const.tile([P, 1], mybir.dt.int16)
            nc.vector.memset(mask_tile[:], -32768)  # 0x8000 bf16 sign mask

            # ---- phase 1: build w tiles [P, OC] bf16, one per k-tile ----
            w_tiles = []
            for kt in range(KT):
                b_tile = bpool.tile([P, OC * K], mybir.dt.bfloat16)
                nc.sync.dma_start(out=b_tile[:], in_=b_re[kt * P:(kt + 1) * P, :])
                # sign bits: b &= 0x8000 (in place)
                nc.vector.tensor_scalar(
                    out=b_tile.bitcast(mybir.dt.int16)[:],
                    in0=b_tile.bitcast(mybir.dt.int16)[:],
                    scalar1=mask_tile[:, 0:1],
                    scalar2=None,
                    op0=mybir.AluOpType.bitwise_and,
                )
                # copysign(a, b) = signbit | a   (a > 0)
                contrib = cpool.tile([P, OC * K], mybir.dt.bfloat16)
                nc.vector.tensor_tensor(
                    out=contrib.bitcast(mybir.dt.int16)[:],
                    in0=b_tile.bitcast(mybir.dt.int16)[:],
                    in1=a_tile.bitcast(mybir.dt.int16)[:],
                    op=mybir.AluOpType.bitwise_or,
                )
                # w = sum over k (innermost, fully dense read)
                w_tile = wpool.tile([P, OC], mybir.dt.bfloat16)
                with nc.allow_low_precision(reason="w = sum of 4 bf16 terms; bf16 matmul input"):
                    nc.vector.tensor_reduce(
                        out=w_tile[:],
                        in_=contrib[:].rearrange("p (o k) -> p o k", k=K),
                        axis=mybir.AxisListType.X,
                        op=mybir.AluOpType.add,
                    )
                w_tiles.append(w_tile)

            # ---- phase 2: out[m] = sum_kt xT[kt,m].T @ w[kt]  (+ bias) ----
            for mb in range(MT // M_BLOCK):
                ms = [mb * M_BLOCK + j for j in range(M_BLOCK)]
                ps_tiles = {m: psum_pool.tile([P, OC], mybir.dt.float32, name=f"ps_{m % M_BLOCK}") for m in ms}
                x_tiles = {}
                for kt in range(KT):
                    xt = xpool.tile([P, P * M_BLOCK], mybir.dt.bfloat16)
                    nc.sync.dma_start(
                        out=xt[:],
                        in_=xT[kt * P:(kt + 1) * P,
                              ms[0] * P:(ms[0] + M_BLOCK) * P],
                    )
                    x_tiles[kt] = xt
                for kt in range(KT):
                    for j, m in enumerate(ms):
                        nc.tensor.matmul(
                            ps_tiles[m][:],
                            x_tiles[kt][:, j * P:(j + 1) * P],
                            w_tiles[kt][:],
                            start=(kt == 0),
                            stop=(kt == KT - 1),
                        )
                for m in ms:
                    o_tile = opool.tile([P, OC], mybir.dt.float32)
                    nc.vector.tensor_tensor(
                        out=o_tile[:], in0=ps_tiles[m][:], in1=bias_tile[:],
                        op=mybir.AluOpType.add,
                    )
                    nc.sync.dma_start(out=out[m * P:(m + 1) * P, :], in_=o_tile[:])

    nc.compile()
    return nc


_NC_CACHE = None


def _get_program():
    global _NC_CACHE
    if _NC_CACHE is None:
        _NC_CACHE = _build_program()
    return _NC_CACHE


def prep_inputs(x, b, a, bias):
    """Host-side shard/cast/layout only. Returns per-core input maps."""
    xT16 = np.ascontiguousarray(x.T).astype(BF16)          # [I, B] bf16
    b16 = b.astype(BF16)                                    # [K, I, O] bf16
    # relayout to [I, O, K] (k innermost) without a giant strided gather:
    b_re_full = np.empty((I, O, K), dtype=BF16)
    for k in range(K):
        b_re_full[:, :, k] = b16[k]
    a16 = np.ascontiguousarray(a.T).astype(BF16)            # [O, K] bf16
    bias32 = bias.astype(np.float32)

    in_maps = []
    for c in range(NCORES):
        sl = slice(c * OC, (c + 1) * OC)
        b_re = np.ascontiguousarray(b_re_full[:, sl, :]).reshape(I, OC * K)
        a_flat = np.ascontiguousarray(a16[sl, :]).reshape(1, OC * K)
        a_bcast = np.broadcast_to(a_flat, (P, OC * K)).copy()
        bias_bcast = np.broadcast_to(bias32[sl].reshape(1, OC), (P, OC)).copy()
        in_maps.append({
            "b_re": b_re,
            "a_b": a_bcast,
            "xT": xT16,
            "bias_b": bias_bcast,
        })
    return in_maps


def run(in_maps, trace=False):
    from concourse.bass_utils import run_bass_kernel_spmd

    nc = _get_program()
    res = run_bass_kernel_spmd(nc, in_maps, list(range(NCORES)), trace=trace)
    return res


def kernel(x, b, a, bias):
    in_maps = prep_inputs(x, b, a, bias)
    res = run(in_maps)
    out = np.concatenate([res.results[c]["out"] for c in range(NCORES)], axis=1)
    return np.ascontiguousarray(out, dtype=np.float32)


if __name__ == "__main__":
    rng = np.random.default_rng(0)
    x = rng.standard_normal((B, I), dtype=np.float32)
    b = rng.standard_normal((K, I, O), dtype=np.float32)
    a = rng.random((K, O), dtype=np.float32)
    bias = rng.standard_normal(O, dtype=np.float32)
    out = kernel(x=x, b=b, a=a, bias=bias)
    w_eff = np.einsum('kio,ko->io', np.sign(b), a.astype(np.float64)).astype(np.float64)
    expected = x.astype(np.float64) @ w_eff + bias
    rel = np.linalg.norm(out - expected) / np.linalg.norm(expected)
    print(f"rel_err = {rel:.3e}")


# revision 4
# speedup vs baseline: 1.0872x; 1.0872x over previous
"""Trainium2 Bass kernel for nn_BinaryDense: out = x @ (sum_k sign(b_k)*a_k) + bias.

Shapes (hardcoded): x [4096,4096] f32, b [4,4096,4096] f32, a [4,4096] f32,
bias [4096] f32 -> out [4096,4096] f32.

Strategy: tensor-parallel over the output (units) dim across 8 NeuronCores.
Core c owns O-columns [c*512, (c+1)*512).

Per core:
  1. Build w[:, oc] = sum_k copysign(a[k,oc], b[k,:,oc]) on-chip.
     b arrives bf16 in [I, K, O_c] layout (k-major), so copysign is two
     bitwise DVE ops ((b & 0x8000) | a, the AND done as packed int32) and
     the k-sum is two fully-dense bf16 adds (DVE 2x mode):
       t = c[0:2] + c[2:4]  (1024-wide), w = t[0] + t[1]  (512-wide).
  2. One bf16 matmul x @ w with fp32 PSUM accumulation:
     lhsT = x^T tiles (host-pretransposed bf16), rhs = w tiles, K=4096.
     8 PSUM banks carry 8 m-tiles through the k loop (k-outer keeps the
     PE fed straight from the w-build pipeline).
  3. Add bias on psum eviction (DVE), store fp32 [4096, 512].

Host side only reshapes/casts/shards (no math): x^T bf16, b -> [I,K,O] bf16,
a/bias broadcast rows.
"""

import sys

if "/opt/trn_rl_repo" not in sys.path:
    sys.path.insert(0, "/opt/trn_rl_repo")

import numpy as np
import ml_dtypes

BF16 = ml_dtypes.bfloat16

B = 4096   # batch rows of x
I = 4096   # input dim (contraction)
O = 4096   # output dim (sharded)
K = 4      # binary bases
NCORES = 8
OC = O // NCORES   # 512 output cols per core
P = 128

KT = I // P        # 32 k-tiles (contraction)
MT = B // P        # 32 m-tiles (output rows)
M_BLOCK = 8        # m-tiles per psum block (8 banks, single-buffered)


def _build_program():
    import concourse.bass as bass
    import concourse.mybir as mybir
    from concourse import bacc
    from concourse.tile import TileContext

    nc = bacc.Bacc(None, target_bir_lowering=False)

    b_re = nc.declare_dram_parameter("b_re", [I, K * OC], mybir.dt.bfloat16, isOutput=False)
    a_b = nc.declare_dram_parameter("a_b", [P, K * OC], mybir.dt.bfloat16, isOutput=False)
    xT = nc.declare_dram_parameter("xT", [I, B], mybir.dt.bfloat16, isOutput=False)
    bias_b = nc.declare_dram_parameter("bias_b", [P, OC], mybir.dt.float32, isOutput=False)
    out = nc.declare_dram_parameter("out", [B, OC], mybir.dt.float32, isOutput=True)

    with TileContext(nc) as tc:
        with (
            tc.tile_pool(name="const", bufs=1) as const,
            tc.tile_pool(name="bpool", bufs=3) as bpool,
            tc.tile_pool(name="cpool", bufs=3) as cpool,
            tc.tile_pool(name="tpool", bufs=3) as tpool,
            tc.tile_pool(name="wpool", bufs=KT) as wpool,
            tc.tile_pool(name="xpool", bufs=4) as xpool,
            tc.tile_pool(name="opool", bufs=3) as opool,
            tc.tile_pool(name="psum", bufs=1, space="PSUM") as psum_pool,
        ):
            a_tile = const.tile([P, K * OC], mybir.dt.bfloat16)
            nc.sync.dma_start(out=a_tile[:], in_=a_b[:, :])
            bias_tile = const.tile([P, OC], mybir.dt.float32)
            nc.sync.dma_start(out=bias_tile[:], in_=bias_b[:, :])
            mask_tile = const.tile([P, 1], mybir.dt.int32)
            nc.vector.memset(mask_tile[:], -2147450880)  # 0x80008000: bf16 sign pair

            # ---- phase 1: build w tiles [P, OC] bf16, one per k-tile ----
            w_tiles = []
            for kt in range(KT):
                b_tile = bpool.tile([P, K * OC], mybir.dt.bfloat16)
                nc.sync.dma_start(out=b_tile[:], in_=b_re[kt * P:(kt + 1) * P, :])
                # sign bits: b &= 0x8000 (in place, packed pairs)
                nc.vector.tensor_scalar(
                    out=b_tile.bitcast(mybir.dt.int32)[:],
                    in0=b_tile.bitcast(mybir.dt.int32)[:],
                    scalar1=mask_tile[:, 0:1],
                    scalar2=None,
                    op0=mybir.AluOpType.bitwise_and,
                )
                # copysign(a, b) = signbit | a   (a > 0)
                contrib = cpool.tile([P, K * OC], mybir.dt.bfloat16)
                nc.vector.tensor_tensor(
                    out=contrib.bitcast(mybir.dt.int16)[:],
                    in0=b_tile.bitcast(mybir.dt.int16)[:],
                    in1=a_tile.bitcast(mybir.dt.int16)[:],
                    op=mybir.AluOpType.bitwise_or,
                )
                # k-sum as two dense bf16 adds (k-major layout)
                t_tile = tpool.tile([P, 2 * OC], mybir.dt.bfloat16)
                nc.vector.tensor_tensor(
                    out=t_tile[:],
                    in0=contrib[:, 0:2 * OC],
                    in1=contrib[:, 2 * OC:4 * OC],
                    op=mybir.AluOpType.add,
                )
                w_tile = wpool.tile([P, OC], mybir.dt.bfloat16)
                nc.vector.tensor_tensor(
                    out=w_tile[:],
                    in0=t_tile[:, 0:OC],
                    in1=t_tile[:, OC:2 * OC],
                    op=mybir.AluOpType.add,
                )
                w_tiles.append(w_tile)

            # ---- phase 2: out[m] = sum_kt xT[kt,m].T @ w[kt]  (+ bias) ----
            for mb in range(MT // M_BLOCK):
                ms = [mb * M_BLOCK + j for j in range(M_BLOCK)]
                ps_tiles = {
                    m: psum_pool.tile([P, OC], mybir.dt.float32, name=f"ps_{m % M_BLOCK}")
                    for m in ms
                }
                for kt in range(KT):
                    xt = xpool.tile([P, P * M_BLOCK], mybir.dt.bfloat16)
                    nc.sync.dma_start(
                        out=xt[:],
                        in_=xT[kt * P:(kt + 1) * P,
                              ms[0] * P:(ms[0] + M_BLOCK) * P],
                    )
                    for j, m in enumerate(ms):
                        nc.tensor.matmul(
                            ps_tiles[m][:],
                            xt[:, j * P:(j + 1) * P],
                            w_tiles[kt][:],
                            start=(kt == 0),
                            stop=(kt == KT - 1),
                        )
                for m in ms:
                    o_tile = opool.tile([P, OC], mybir.dt.float32)
                    nc.vector.tensor_tensor(
                        out=o_tile[:], in0=ps_tiles[m][:], in1=bias_tile[:],
                        op=mybir.AluOpType.add,
                    )
                    nc.sync.dma_start(out=out[m * P:(m + 1) * P, :], in_=o_tile[:])

    nc.compile()
    return nc


_NC_CACHE = None


def _get_program():
    global _NC_CACHE
    if _NC_CACHE is None:
        _NC_CACHE = _build_program()
    return _NC_CACHE


def prep_inputs(x, b, a, bias):
    """Host-side shard/cast/layout only. Returns per-core input maps."""
    xT16 = np.ascontiguousarray(x.T).astype(BF16)          # [I, B] bf16
    b_iko = np.transpose(b, (1, 0, 2)).astype(BF16)        # [I, K, O] bf16
    bias32 = bias.astype(np.float32)
    a16 = a.astype(BF16)                                    # [K, O]

    in_maps = []
    for c in range(NCORES):
        sl = slice(c * OC, (c + 1) * OC)
        b_slice = np.ascontiguousarray(b_iko[:, :, sl]).reshape(I, K * OC)
        a_flat = np.ascontiguousarray(a16[:, sl]).reshape(1, K * OC)
        a_bcast = np.broadcast_to(a_flat, (P, K * OC)).copy()
        bias_bcast = np.broadcast_to(bias32[sl].reshape(1, OC), (P, OC)).copy()
        in_maps.append({
            "b_re": b_slice,
            "a_b": a_bcast,
            "xT": xT16,
            "bias_b": bias_bcast,
        })
    return in_maps


def run(in_maps, trace=False):
    from concourse.bass_utils import run_bass_kernel_spmd

    nc = _get_program()
    res = run_bass_kernel_spmd(nc, in_maps, list(range(NCORES)), trace=trace)
    return res


def kernel(x, b, a, bias):
    in_maps = prep_inputs(x, b, a, bias)
    res = run(in_maps)
    out = np.concatenate([res.results[c]["out"] for c in range(NCORES)], axis=1)
    return np.ascontiguousarray(out, dtype=np.float32)


if __name__ == "__main__":
    rng = np.random.default_rng(0)
    x = rng.standard_normal((B, I), dtype=np.float32)
    b = rng.standard_normal((K, I, O), dtype=np.float32)
    a = rng.random((K, O), dtype=np.float32)
    bias = rng.standard_normal(O, dtype=np.float32)
    out = kernel(x=x, b=b, a=a, bias=bias)
    w_eff = np.einsum('kio,ko->io', np.sign(b), a.astype(np.float64)).astype(np.float64)
    expected = x.astype(np.float64) @ w_eff + bias
    rel = np.linalg.norm(out - expected) / np.linalg.norm(expected)
    print(f"rel_err = {rel:.3e}")


# revision 7
# speedup vs baseline: 1.1530x; 1.0605x over previous
"""Trainium2 Bass kernel for nn_BinaryDense: out = x @ (sum_k sign(b_k)*a_k) + bias.

Shapes (hardcoded): x [4096,4096] f32, b [4,4096,4096] f32, a [4,4096] f32,
bias [4096] f32 -> out [4096,4096] f32.

Strategy: tensor-parallel over the output (units) dim across 8 NeuronCores.
Core c owns O-columns [c*512, (c+1)*512).

Per core:
  1. Build w[:, oc] = sum_k copysign(a[k,oc], b[k,:,oc]) on-chip.
     b arrives bf16 in [I, K, O_c] layout (k-major), so copysign is two
     bitwise DVE ops ((b & 0x8000) | a, the AND done as packed int32) and
     the k-sum is two fully-dense bf16 adds (DVE 2x mode):
       t = c[0:2] + c[2:4]  (1024-wide), w = t[0] + t[1]  (512-wide).
  2. One bf16 matmul x @ w with fp32 PSUM accumulation:
     lhsT = x^T tiles (host-pretransposed bf16), rhs = w tiles, K=4096.
     8 PSUM banks carry 8 m-tiles through the k loop (k-outer keeps the
     PE fed straight from the w-build pipeline).
  3. Add bias on psum eviction (DVE), store fp32 [4096, 512].

Host side only reshapes/casts/shards (no math): x^T bf16, b -> [I,K,O] bf16,
a/bias broadcast rows.
"""

import sys

if "/opt/trn_rl_repo" not in sys.path:
    sys.path.insert(0, "/opt/trn_rl_repo")

import numpy as np
import ml_dtypes

BF16 = ml_dtypes.bfloat16

B = 4096   # batch rows of x
I = 4096   # input dim (contraction)
O = 4096   # output dim (sharded)
K = 4      # binary bases
NCORES = 8
OC = O // NCORES   # 512 output cols per core
P = 128

KT = I // P        # 32 k-tiles (contraction)
MT = B // P        # 32 m-tiles (output rows)
M_BLOCK = 8        # m-tiles per psum block (8 banks, single-buffered)


def _build_program():
    import concourse.bass as bass
    import concourse.mybir as mybir
    from concourse import bacc
    from concourse.tile import TileContext

    nc = bacc.Bacc(None, target_bir_lowering=False)

    b_re = nc.declare_dram_parameter("b_re", [I, K * OC], mybir.dt.bfloat16, isOutput=False)
    a_b = nc.declare_dram_parameter("a_b", [P, K * OC], mybir.dt.bfloat16, isOutput=False)
    xT = nc.declare_dram_parameter("xT", [I, B], mybir.dt.bfloat16, isOutput=False)
    bias_b = nc.declare_dram_parameter("bias_b", [P, OC], mybir.dt.float32, isOutput=False)
    out = nc.declare_dram_parameter("out", [B, OC], mybir.dt.float32, isOutput=True)

    with TileContext(nc) as tc:
        with (
            tc.tile_pool(name="const", bufs=1) as const,
            tc.tile_pool(name="bpool", bufs=3) as bpool,
            tc.tile_pool(name="cpool", bufs=3) as cpool,
            tc.tile_pool(name="tpool", bufs=3) as tpool,
            tc.tile_pool(name="wpool", bufs=KT) as wpool,
            tc.tile_pool(name="xpool", bufs=8) as xpool,
            tc.tile_pool(name="opool", bufs=3) as opool,
            tc.tile_pool(name="psum", bufs=1, space="PSUM") as psum_pool,
        ):
            a_tile = const.tile([P, K * OC], mybir.dt.bfloat16)
            nc.sync.dma_start(out=a_tile[:], in_=a_b[:, :])
            bias_tile = const.tile([P, OC], mybir.dt.float32)
            nc.sync.dma_start(out=bias_tile[:], in_=bias_b[:, :])
            mask_tile = const.tile([P, 1], mybir.dt.int16)
            nc.vector.memset(mask_tile[:], -32768)  # 0x8000: bf16 sign mask

            # ---- phase 1: build w tiles [P, OC] bf16, one per k-tile ----
            w_tiles = []
            for kt in range(KT):
                b_tile = bpool.tile([P, K * OC], mybir.dt.bfloat16)
                nc.sync.dma_start(out=b_tile[:], in_=b_re[kt * P:(kt + 1) * P, :])
                # copysign(a, b) = (b & 0x8000) | a  (a > 0), one fused DVE op
                contrib = cpool.tile([P, K * OC], mybir.dt.bfloat16)
                nc.vector.scalar_tensor_tensor(
                    out=contrib.bitcast(mybir.dt.int16)[:],
                    in0=b_tile.bitcast(mybir.dt.int16)[:],
                    scalar=mask_tile[:, 0:1],
                    in1=a_tile.bitcast(mybir.dt.int16)[:],
                    op0=mybir.AluOpType.bitwise_and,
                    op1=mybir.AluOpType.bitwise_or,
                )
                # k-sum as two dense bf16 adds (k-major layout)
                t_tile = tpool.tile([P, 2 * OC], mybir.dt.bfloat16)
                nc.vector.tensor_tensor(
                    out=t_tile[:],
                    in0=contrib[:, 0:2 * OC],
                    in1=contrib[:, 2 * OC:4 * OC],
                    op=mybir.AluOpType.add,
                )
                w_tile = wpool.tile([P, OC], mybir.dt.bfloat16)
                nc.vector.tensor_tensor(
                    out=w_tile[:],
                    in0=t_tile[:, 0:OC],
                    in1=t_tile[:, OC:2 * OC],
                    op=mybir.AluOpType.add,
                )
                w_tiles.append(w_tile)

            # ---- phase 2: out[m] = sum_kt xT[kt,m].T @ w[kt]  (+ bias) ----
            for mb in range(MT // M_BLOCK):
                ms = [mb * M_BLOCK + j for j in range(M_BLOCK)]
                ps_tiles = {
                    m: psum_pool.tile([P, OC], mybir.dt.float32, name=f"ps_{m % M_BLOCK}")
                    for m in ms
                }
                for kt in range(KT):
                    xt = xpool.tile([P, P * M_BLOCK], mybir.dt.bfloat16)
                    nc.sync.dma_start(
                        out=xt[:],
                        in_=xT[kt * P:(kt + 1) * P,
                              ms[0] * P:(ms[0] + M_BLOCK) * P],
                    )
                    for j, m in enumerate(ms):
                        nc.tensor.matmul(
                            ps_tiles[m][:],
                            xt[:, j * P:(j + 1) * P],
                            w_tiles[kt][:],
                            start=(kt == 0),
                            stop=(kt == KT - 1),
                        )
                for m in ms:
                    o_tile = opool.tile([P, OC], mybir.dt.float32)
                    nc.vector.tensor_tensor(
                        out=o_tile[:], in0=ps_tiles[m][:], in1=bias_tile[:],
                        op=mybir.AluOpType.add,
                    )
                    nc.gpsimd.dma_start(out=out[m * P:(m + 1) * P, :], in_=o_tile[:])

    nc.compile()
    return nc


_NC_CACHE = None


def _get_program():
    global _NC_CACHE
    if _NC_CACHE is None:
        _NC_CACHE = _build_program()
    return _NC_CACHE


def prep_inputs(x, b, a, bias):
    """Host-side shard/cast/layout only. Returns per-core input maps."""
    xT16 = np.ascontiguousarray(x.T).astype(BF16)          # [I, B] bf16
    b_iko = np.transpose(b, (1, 0, 2)).astype(BF16)        # [I, K, O] bf16
    bias32 = bias.astype(np.float32)
    a16 = a.astype(BF16)                                    # [K, O]

    in_maps = []
    for c in range(NCORES):
        sl = slice(c * OC, (c + 1) * OC)
        b_slice = np.ascontiguousarray(b_iko[:, :, sl]).reshape(I, K * OC)
        a_flat = np.ascontiguousarray(a16[:, sl]).reshape(1, K * OC)
        a_bcast = np.broadcast_to(a_flat, (P, K * OC)).copy()
        bias_bcast = np.broadcast_to(bias32[sl].reshape(1, OC), (P, OC)).copy()
        in_maps.append({
            "b_re": b_slice,
            "a_b": a_bcast,
            "xT": xT16,
            "bias_b": bias_bcast,
        })
    return in_maps


def run(in_maps, trace=False):
    from concourse.bass_utils import run_bass_kernel_spmd

    nc = _get_program()
    res = run_bass_kernel_spmd(nc, in_maps, list(range(NCORES)), trace=trace)
    return res


def kernel(x, b, a, bias):
    in_maps = prep_inputs(x, b, a, bias)
    res = run(in_maps)
    out = np.concatenate([res.results[c]["out"] for c in range(NCORES)], axis=1)
    return np.ascontiguousarray(out, dtype=np.float32)


if __name__ == "__main__":
    rng = np.random.default_rng(0)
    x = rng.standard_normal((B, I), dtype=np.float32)
    b = rng.standard_normal((K, I, O), dtype=np.float32)
    a = rng.random((K, O), dtype=np.float32)
    bias = rng.standard_normal(O, dtype=np.float32)
    out = kernel(x=x, b=b, a=a, bias=bias)
    w_eff = np.einsum('kio,ko->io', np.sign(b), a.astype(np.float64)).astype(np.float64)
    expected = x.astype(np.float64) @ w_eff + bias
    rel = np.linalg.norm(out - expected) / np.linalg.norm(expected)
    print(f"rel_err = {rel:.3e}")
